# revision 18
# baseline (speedup 1.0000x reference)
"""Trainium2 Bass kernel for nn_AttentionHead (B=4, S=4096, D=512).

reference:
    K = x @ Wk.T; Q = x @ Wq.T; V = x @ Wv.T            # [B,S,D]
    scores[b,s,t] = <K[b,s], Q[b,t]> / sqrt(D)
    scores[b,:,t] = -1e12 where mask[b,t]==0
    out = softmax(scores, axis=t) @ V                    # [B,S,D]

Sharding: 8 cores = 4 batches x 2 sequence halves (rows s of the score
matrix). No collectives (2-core collective measured ~40GB/s -- slower
than recomputing the projections).

Algorithmic structure (see kernel_f32r_backup.py for the f32r ancestor):

1. Key compaction: masked keys contribute EXACTLY zero, so the host
   gathers only the ~50% unmasked key columns (pure indexing).  Pad
   positions carry mask=0 so their exp bias (-1e9) zeroes them.

2. Projection fusion: scores = x (Wk^T Wq) x^T.  A := Wk^T Wq, then
   G := x_half A, scores = G @ x_kept^T -- the Q projection disappears.

3. All matmul OPERANDS in fp16 (psum accumulation stays f32): HW
   microbench shows 16-bit passes run at 215.8ns vs f32r 226.7ns at
   FD=512 (FWL: LDWEIGHTS 97ns vs 187ns, fully hidden), and input DMA
   bytes halve, which removes most phase-1 DMA stalls.  fp16 (e5m10)
   beats bf16 here: 4x less quantization noise (end-to-end 1.5e-3 vs
   9.6e-3 measured) at identical speed, and the margin lets the OUTPUT
   ship as f16 too (host upcasts), halving the final DMA flush.
   fp8 (DoubleRow, 2x contraction/pass) was measured at 215.8ns/pass
   -- a true 2x -- but ANY fp8 operand (even only P or only V) pushes
   the error to >=2.5e-2 (gate 2e-2): dead on accuracy, not speed.

Schedule (per core), TK = padded kept-key count.  DMA priority: wk+wq
(A's operands), first xk tile, wv, all of xq, rest of xk.  PE program
order: warm-up, A, G, then the attention chunks; the V projection is
INLINED tile-by-tile into the first s-chunk's t-loop so it paces with
the xk DMA stream:
    per s-chunk of 512, for each kept t-tile of 128:
        [sc==0 only] V[t,:] = x_k-tile.T @ Wv^T-tiles   (PSUM, 4 MMs)
        S^T[t,s]  = sum_d x_k^T-tile.T @ G^T            (PSUM, 4 MMs)
        P^T       = exp(S^T/sqrt(D) + mbias[t])         (ACT -> f16)
        out^T[d,s]+= V-tile.T @ P^T                     (4 MMs, PSUM acc)
        den128    += P^T                                (DVE, f32 accum)
    epilogue (DEFERRED into the next chunk so the PE never waits on
    the recip chain): den = ones.T @ den128 (+ f16-ones @ last P^T),
    fast reciprocal, broadcast via a rank-1 matmul, out^T *= 1/den,
    DMA out^T (f16).  The last two s-chunks are 256 wide to halve the
    exposed final epilogue + output flush.

Host passes x^T / W layouts in f16 (pure permutations/gathers + dtype
cast; all FLOPs stay on device); host upcasts the f16 outT to f32.

Measured: ~181.6us HW exec (f32r ancestor: ~194.3us; harness baseline
235.3us).  PE is back-to-back at 215.8ns/pass in steady state; the
rest is ~7.2us sequencer boot + clock ramp + ~4us phase-1 DMA floor +
~6us end-of-NEFF output flush/drain.  (Device clock wanders between
sessions: some runs measure uniformly ~+18%; take best-of-N.)
"""

import numpy as np

import concourse.bacc as bacc
import concourse.mybir as mybir
from concourse.bass_utils import run_bass_kernel_spmd
from concourse.tile import TileContext

B, S, D = 4, 4096, 512
SH = S // 2          # per-core s rows (half sequence)
P = 128              # partition tile
CH = 512             # free-dim chunk
KD = D // P          # 4 contraction tiles over d
SCALE = 1.0 / float(np.sqrt(D))

F32 = mybir.dt.float32
F32R = mybir.dt.float32r
F16 = mybir.dt.float16
COPY = mybir.ActivationFunctionType.Copy
EXP = mybir.ActivationFunctionType.Exp
NPF16 = np.float16

_CACHE = {}


def _build(TK):
    NTK = TK // P        # kept-key tiles
    nc = bacc.Bacc(num_devices=8)
    # all inputs host-reshaped to [P, KD, *] so each tensor lands in 1-4
    # DMA instructions -- DMA-issue instructions cost ~600ns of issuing-
    # engine queue time each, and a jammed queue stalls the PE's psum
    # copies behind them
    xkT = nc.declare_dram_parameter("xkT", [P, KD, TK], F16, isOutput=False)
    xqT = nc.declare_dram_parameter("xqT", [P, KD, SH], F16, isOutput=False)
    wkN = nc.declare_dram_parameter("wkN", [P, KD, D], F16, isOutput=False)
    wqN = nc.declare_dram_parameter("wqN", [P, KD, D], F16, isOutput=False)
    wvT = nc.declare_dram_parameter("wvT", [P, KD, D], F16, isOutput=False)
    maskT = nc.declare_dram_parameter("maskT", [P, NTK], F32, isOutput=False)
    outT = nc.declare_dram_parameter("outT", [D, SH], F16, isOutput=True)

    engs = None

    with TileContext(nc) as tc:
        with tc.tile_pool(name="pers", bufs=1) as pers:
            xk = pers.tile([P, KD, TK], F16)     # x^T kept keys (d-tiled)
            gT = pers.tile([P, KD, SH], F16)     # G^T local half
            vA = pers.tile([P, NTK, D], F16)     # V kept keys (t-tiled)
            wv = pers.tile([P, KD, D], F16)      # Wv^T (V inlined in sc=0)
            mk = pers.tile([P, NTK], F32)
            mbias = pers.tile([P, NTK], F32)
            ones = pers.tile([4, P], F16)
            ones32 = pers.tile([4, P], F32)
            recr4 = pers.tile([4, CH], F16)      # row 0 = 1/den, rows 1-3
                                                  # stay zero (widens the
                                                  # bps contraction: K=1
                                                  # matmuls run ~440ns)
            onec = pers.tile([P, 4], F32R)
            onecb = pers.tile([P, 4], F16)       # f16 ones for the pt
                                                  # colsum leg of dps
            onec32 = pers.tile([P, 4], F32)

            # phase-1 staging tiles live in the persistent pool: SBUF
            # fits (~10MB of 24) and a separate pool's exit barrier would
            # stall the PE between G and the first v_group
            wk = pers.tile([P, KD, D], F16)
            wq = pers.tile([P, KD, D], F16)
            xq = pers.tile([P, KD, SH], F16)
            aSb = pers.tile([P, KD, D], F16)
            warm32 = pers.tile([P, CH], F32)

            # ---------------- phase 1: A, G ----------------
            with tc.tile_pool(name="ppsum", bufs=2, space="PSUM") as ppsum:

                # PE warm-up while the first DMAs land (keeps the HAM
                # clock-gate from dropping the PE to half rate).  The warm
                # operand is memset on GPSIMD (wakes ~1us before Vector
                # after the program load) and stays plain F32: an fp32
                # matmul runs 4 cycles/row, so 6 passes cover the ~4us
                # ramp window with no f32r conversion dependency.
                nc.gpsimd.memset(warm32, 0.0)
                for r in range(6):
                    wps = ppsum.tile([P, CH], F32, tag="warm", bufs=2,
                                     name="wps")
                    nc.tensor.matmul(wps, warm32[:, 0:P], warm32,
                                     start=True, stop=True)

                # DMA issue on sync/gpsimd ONLY -- scalar must stay free
                # for the psum->SBUF copies the PE pipeline depends on.
                # Ring split by DEADLINE: sync carries the G-path stream
                # (wk/wq halves, then all xq chunks in consumption order,
                # then the first xk tile), gpsimd carries the V-path
                # (other wk/wq halves, wv -- needed when v_group(0) runs
                # at ~26us -- then the xk chunk stream).
                engs = [nc.sync, nc.gpsimd]
                nc.sync.dma_start(out=mk, in_=maskT[:, :])
                nc.scalar.dma_start(out=wv, in_=wvT[:, :, :])
                nc.sync.dma_start(out=wk[:, 0:2, :], in_=wkN[:, 0:2, :])
                nc.gpsimd.dma_start(out=wk[:, 2:4, :], in_=wkN[:, 2:4, :])
                nc.sync.dma_start(out=wq[:, 0:2, :], in_=wqN[:, 0:2, :])
                nc.gpsimd.dma_start(out=wq[:, 2:4, :], in_=wqN[:, 2:4, :])
                for c in range(SH // CH):
                    nc.sync.dma_start(
                        out=xq[:, 0:2, c * CH:(c + 1) * CH],
                        in_=xqT[:, 0:2, c * CH:(c + 1) * CH])
                    nc.gpsimd.dma_start(
                        out=xq[:, 2:4, c * CH:(c + 1) * CH],
                        in_=xqT[:, 2:4, c * CH:(c + 1) * CH])
                nc.sync.dma_start(out=xk[:, :, 0:P], in_=xkT[:, :, 0:P])
                ci = P
                nch = 0
                while ci < TK:
                    hi = min(ci + CH, TK)
                    engs[nch % 2].dma_start(out=xk[:, :, ci:hi],
                                            in_=xkT[:, :, ci:hi])
                    ci = hi
                    nch += 1

                # constants + pad-mask bias
                nc.vector.memset(ones32, 1.0)
                nc.vector.tensor_copy(out=ones, in_=ones32)
                nc.vector.tensor_copy(out=recr4, in_=warm32[0:4, :])
                nc.vector.memset(onec32, 1.0)
                nc.vector.tensor_copy(out=onec, in_=onec32)
                nc.vector.tensor_copy(out=onecb, in_=onec32)
                # mbias = (padmask-1)*1e9: 0 kept, -1e9 pad -> exp == 0
                nc.gpsimd.tensor_scalar(mbias, mk, -1.0, 1.0e9,
                                        mybir.AluOpType.add,
                                        mybir.AluOpType.mult)

                # A = Wk^T Wq  (psum i-chunk io -> aSb[:, io, :])
                for io in range(KD):
                    pa = ppsum.tile([P, CH], F32, tag="pa", name="pa")
                    for mt in range(KD):
                        nc.tensor.matmul(
                            pa,
                            wk[:, mt, io * P:(io + 1) * P],
                            wq[:, mt, :],
                            start=(mt == 0), stop=(mt == KD - 1))
                    if io % 2 == 0:
                        nc.scalar.activation(out=aSb[:, io, :], in_=pa,
                                             func=COPY)
                    else:
                        nc.vector.tensor_copy(out=aSb[:, io, :], in_=pa)

                # G^T = A-contracted x_half^T (a single matmul cannot
                # write wider than one 512-f32 PSUM bank)
                for c in range(SH // CH):
                    for jo in range(KD):
                        pg = ppsum.tile([P, CH], F32, tag="pg", name="pg")
                        for it in range(KD):
                            nc.tensor.matmul(
                                pg,
                                aSb[:, it, jo * P:(jo + 1) * P],
                                xq[:, it, c * CH:(c + 1) * CH],
                                start=(it == 0), stop=(it == KD - 1))
                        if jo % 2 == 0:
                            nc.scalar.activation(
                                out=gT[:, jo, c * CH:(c + 1) * CH], in_=pg,
                                func=COPY)
                        else:
                            nc.vector.tensor_copy(
                                out=gT[:, jo, c * CH:(c + 1) * CH], in_=pg)

            # ------------- phase 2: attention (V inlined in sc=0) -------
            with tc.tile_pool(name="att", bufs=1) as att, \
                 tc.tile_pool(name="apsum", bufs=1, space="PSUM") as apsum:

                def v_group(ti):
                    # V[t-tile ti] = x_k-tile.T @ Wv^T; psum shares the
                    # "bc" bank (den/broadcast only run after the last
                    # v_group of the chunk)
                    pv = apsum.tile([P, D], F32, tag="bc", name="pv")
                    for kd in range(KD):
                        nc.tensor.matmul(
                            pv,
                            xk[:, kd, ti * P:(ti + 1) * P],
                            wv[:, kd, :],
                            start=(kd == 0), stop=(kd == KD - 1))
                    nc.scalar.activation(out=vA[:, ti, :], in_=pv, func=COPY)

                # Per-chunk epilogues (broadcast + normalize + DMA) for
                # non-final chunks are DEFERRED into the next chunk: the
                # bps matmul otherwise sits in the PE stream waiting on
                # the DVE recip chain at every chunk boundary.  The
                # deferred block is emitted right after the next chunk's
                # first two s_groups, by which point recr4 is ready.
                # The final two chunks are 256 wide so the last chunk's
                # exposed epilogue + output flush is half as long.
                deferred = [None]
                CHUNKS = [(0, CH), (CH, CH), (2 * CH, CH),
                          (3 * CH, CH // 2), (3 * CH + CH // 2, CH // 2)]

                def emit_epilogue(off, chw, opsum, osb, dps, last):
                    # recip needs f32 in/out (bit-level seed), then a tiny
                    # staging copy into the f16 broadcast operand
                    rec = att.tile([1, CH], F32, tag="rec")
                    nc.vector.reciprocal_approx_fast(out=rec[:, 0:chw],
                                                     in_=dps[0:1, 0:chw])
                    nc.vector.tensor_copy(out=recr4[0:1, 0:chw],
                                          in_=rec[:, 0:chw])
                    bps = apsum.tile([P, CH], F32, tag="bc", name="bps")
                    nc.tensor.matmul(bps[:, 0:chw], ones, recr4[:, 0:chw],
                                     start=True, stop=True)
                    bsb = att.tile([P, CH], F32, tag="bsb")
                    nc.vector.tensor_copy(out=bsb[:, 0:chw],
                                          in_=bps[:, 0:chw])
                    for d in (3, 0, 1, 2) if last else range(KD):
                        fin = att.tile([P, CH], F16, tag=f"fin{d % 2}",
                                       name=f"fin{d}", bufs=2)
                        if last:
                            # multiply straight out of PSUM; GPSIMD can't
                            # read PSUM and its multiplies run ~2x slower
                            # than DVE, so DVE takes 3 of the 4 windows
                            # and GPSIMD one (via a scalar COPY drain,
                            # started first: it's the longest chain)
                            if d != 3:
                                nc.vector.tensor_mul(fin[:, 0:chw],
                                                     opsum[d][:, 0:chw],
                                                     bsb[:, 0:chw])
                            else:
                                ot = att.tile([P, CH], F32, tag=f"osb{d}",
                                              name=f"osb{d}")
                                nc.scalar.activation(out=ot[:, 0:chw],
                                                     in_=opsum[d][:, 0:chw],
                                                     func=COPY)
                                nc.gpsimd.tensor_mul(fin[:, 0:chw],
                                                     ot[:, 0:chw],
                                                     bsb[:, 0:chw])
                        else:
                            meng = nc.vector if d % 2 == 0 else nc.gpsimd
                            meng.tensor_mul(fin[:, 0:chw], osb[d][:, 0:chw],
                                            bsb[:, 0:chw])
                        eng = engs[d % 2]
                        eng.dma_start(
                            out=outT[d * P:(d + 1) * P, off:off + chw],
                            in_=fin[:, 0:chw])

                for ci, (off, chw) in enumerate(CHUNKS):
                    opsum = [apsum.tile([P, CH], F32, tag=f"o{d}",
                                        name=f"opsum{d}")
                             for d in range(KD)]
                    den128 = att.tile([P, CH], F32R, tag="den128")

                    def s_group(ti, off=off, chw=chw):
                        ss = apsum.tile([P, CH], F32, tag="s", bufs=3)
                        for kd in range(KD):
                            nc.tensor.matmul(
                                ss[:, 0:chw],
                                xk[:, kd, ti * P:(ti + 1) * P],
                                gT[:, kd, off:off + chw],
                                start=(kd == 0), stop=(kd == KD - 1))
                        return ss

                    last = (ci == len(CHUNKS) - 1)
                    if ci == 0:
                        v_group(0)
                    ss_cur = s_group(0)
                    for ti in range(NTK):
                        if ci == 0 and ti + 1 < NTK:
                            v_group(ti + 1)
                        ss_next = s_group(ti + 1) if ti + 1 < NTK else None
                        dpt = 1 if chw == CH else 2
                        if ti == dpt and deferred[0] is not None:
                            # previous chunk's broadcast/normalize: its
                            # recip chain has had two s_groups to finish
                            deferred[0]()
                            deferred[0] = None
                        pt = att.tile([P, CH], F16, tag="pt", bufs=3)
                        # pad-masked softmax numerator
                        nc.scalar.activation(out=pt[:, 0:chw],
                                             in_=ss_cur[:, 0:chw], func=EXP,
                                             scale=SCALE,
                                             bias=mbias[:, ti:ti + 1])
                        if ti == NTK - 1:
                            # den colsum early -- partial den128 while
                            # ACT runs the final EXP, the final tile's
                            # pt straight into the psum (f16-ones leg)
                            # -- so the reciprocal chain hides under the
                            # final PV group instead of stalling the
                            # broadcast
                            dps = apsum.tile([4, CH], F32, tag="bc",
                                             name="dps")
                            nc.tensor.matmul(dps[:, 0:chw], onec,
                                             den128[:, 0:chw],
                                             start=True, stop=False)
                            nc.tensor.matmul(dps[:, 0:chw], onecb,
                                             pt[:, 0:chw],
                                             start=False, stop=True)
                        for d in range(KD):
                            nc.tensor.matmul(
                                opsum[d][:, 0:chw],
                                vA[:, ti, d * P:(d + 1) * P],
                                pt[:, 0:chw],
                                start=(ti == 0), stop=(ti == NTK - 1))
                        if ti == 0:
                            nc.vector.tensor_copy(out=den128[:, 0:chw],
                                                  in_=pt[:, 0:chw])
                        elif ti != NTK - 1:
                            nc.vector.tensor_add(den128[:, 0:chw],
                                                 den128[:, 0:chw],
                                                 pt[:, 0:chw])
                        ss_cur = ss_next

                    if not last:
                        # drain psum banks via DVE so the PE can start
                        # the next chunk's PV groups without waiting;
                        # order (0,3,1,2) matches the PE's arrival order
                        # at the next chunk's first PV group (d3's drain
                        # must not be last or PV0/d3 stalls on the bank)
                        osb = [None] * KD
                        for d in (0, 3, 1, 2):
                            ot = att.tile([P, CH], F32, tag=f"osb{d}",
                                          name=f"osb{d}")
                            nc.vector.tensor_copy(out=ot[:, 0:chw],
                                                  in_=opsum[d][:, 0:chw])
                            osb[d] = ot
                        deferred[0] = (lambda off=off, chw=chw,
                                       opsum=opsum, osb=osb, dps=dps:
                                       emit_epilogue(off, chw, opsum, osb,
                                                     dps, False))
                    else:
                        emit_epilogue(off, chw, opsum, None, dps, True)

    nc.compile()
    return nc


def _pkd(a):
    """[D, X] -> [P, KD, X] bf16: partition-major d-tiling."""
    return np.ascontiguousarray(
        a.reshape(KD, P, a.shape[1]).transpose(1, 0, 2)).astype(NPF16)


def make_in_maps(x, mask, Wk, Wq, Wv):
    """Host-side prep: per-core input dict. Pure permutations/gathers."""
    x = np.asarray(x, dtype=np.float32)
    mask = np.asarray(mask)
    wkN = _pkd(np.asarray(Wk, dtype=np.float32))
    wqN = _pkd(np.asarray(Wq, dtype=np.float32))
    wvT = _pkd(np.asarray(Wv, dtype=np.float32).T)

    idxs = [np.flatnonzero(mask[b]) for b in range(B)]
    TK = ((max(len(i) for i in idxs) + P - 1) // P) * P
    NTK = TK // P

    in_maps = []
    for b in range(B):
        idx = idxs[b]
        xkT = np.zeros((D, TK), dtype=np.float32)
        xkT[:, :len(idx)] = x[b][idx].T
        xkT = _pkd(xkT)
        padmask = np.zeros(TK, dtype=np.float32)
        padmask[:len(idx)] = 1.0
        maskT = np.ascontiguousarray(padmask.reshape(NTK, P).T)
        xTb = x[b].T
        for h in range(2):
            in_maps.append({
                "xkT": xkT,
                "xqT": _pkd(xTb[:, h * SH:(h + 1) * SH]),
                "wkN": wkN, "wqN": wqN, "wvT": wvT,
                "maskT": maskT,
            })
    return in_maps, TK


def kernel(x, mask, Wk, Wq, Wv):
    in_maps, TK = make_in_maps(x, mask, Wk, Wq, Wv)
    if ("nc", TK) not in _CACHE:
        _CACHE[("nc", TK)] = _build(TK)
        _CACHE["nc"] = _CACHE[("nc", TK)]   # convenience handle
    nc = _CACHE[("nc", TK)]

    res = run_bass_kernel_spmd(nc, in_maps, core_ids=list(range(8)))

    out = np.empty((B, S, D), dtype=np.float32)
    for b in range(B):
        for h in range(2):
            out[b, h * SH:(h + 1) * SH, :] = np.asarray(
                res.results[2 * b + h]["outT"]).astype(np.float32).T
    return out
